# revision 4
# baseline (speedup 1.0000x reference)
"""CenterLoss kernel for Trainium2 (Bass/Tile), data-parallel over 8 NeuronCores.

loss = 0.5 * sum_i ||x_i - centers[targets_i]||^2

The reference materializes the full [N, C] distance matrix and gathers one
entry per row; here we gather only the target center rows (indirect DMA) and
do a fused subtract / square-accumulate, so the kernel is memory-bound on
~4 MB of HBM traffic per core instead of a 69 GFLOP matmul.

Sharding: inputs/targets split along batch N across 8 cores (512 rows each),
centers replicated. Each core returns 128 per-partition partial sums; host
sums them and scales by 0.5.
"""

import numpy as np

import concourse.bacc as bacc
import concourse.bass as bass
import concourse.tile as tile
from concourse import mybir
from concourse.bass_utils import run_bass_kernel_spmd

N, C, D = 4096, 8192, 1024
N_CORES = 8
ROWS = N // N_CORES  # 512 rows per core
P = 128              # SBUF partitions
CHUNKS = ROWS // P   # 4 chunks of 128 rows

# Stashed BassKernelResults from the most recent kernel() call (for profiling).
LAST_RESULTS = None
_NC_CACHE = None


def _build_bass():
    nc = bacc.Bacc("TRN2", target_bir_lowering=False)
    x = nc.dram_tensor("x", [ROWS, D], mybir.dt.float32, kind="ExternalInput")
    idx = nc.dram_tensor("idx", [P, CHUNKS], mybir.dt.int32, kind="ExternalInput")
    centers = nc.dram_tensor("centers", [C, D], mybir.dt.float32, kind="ExternalInput")
    out = nc.dram_tensor("out", [P, 1], mybir.dt.float32, kind="ExternalOutput")

    # Row r of the shard lives at partition p = r // CHUNKS, chunk t = r % CHUNKS,
    # so each partition's chunk reads 4 KB contiguous from DRAM.
    x_pt = x.rearrange("(p t) d -> p t d", p=P)

    with tile.TileContext(nc) as tc:
        with (
            tc.tile_pool(name="io", bufs=CHUNKS) as io,
            tc.tile_pool(name="small", bufs=1) as small,
        ):
            idx_sb = small.tile([P, CHUNKS], mybir.dt.int32)
            nc.sync.dma_start(idx_sb[:], idx[:, :])
            acc = small.tile([P, CHUNKS], mybir.dt.float32)
            for t in range(CHUNKS):
                xt = io.tile([P, D], mybir.dt.float32, tag="x")
                nc.sync.dma_start(xt[:], x_pt[:, t, :])
                ct = io.tile([P, D], mybir.dt.float32, tag="c")
                nc.gpsimd.indirect_dma_start(
                    out=ct[:],
                    out_offset=None,
                    in_=centers[:, :],
                    in_offset=bass.IndirectOffsetOnAxis(
                        ap=idx_sb[:, t : t + 1], axis=0
                    ),
                )
                # d = x - c (in place over the gathered centers)
                nc.vector.tensor_sub(ct[:], xt[:], ct[:])
                # acc[:, t] = sum_d d^2 (ACT engine, fused square + row-sum)
                nc.scalar.activation(
                    out=ct[:],
                    in_=ct[:],
                    func=mybir.ActivationFunctionType.Square,
                    accum_out=acc[:, t : t + 1],
                )
            s = small.tile([P, 1], mybir.dt.float32)
            nc.vector.tensor_reduce(
                out=s[:], in_=acc[:], axis=mybir.AxisListType.X, op=mybir.AluOpType.add
            )
            nc.sync.dma_start(out[:, :], s[:])
    nc.finalize()
    return nc


def _get_nc():
    global _NC_CACHE
    if _NC_CACHE is None:
        _NC_CACHE = _build_bass()
    return _NC_CACHE


def kernel(inputs, targets, centers):
    global LAST_RESULTS
    x = np.ascontiguousarray(np.asarray(inputs, dtype=np.float32))
    tgt = np.asarray(targets).astype(np.int32)
    cen = np.ascontiguousarray(np.asarray(centers, dtype=np.float32))
    assert x.shape == (N, D) and cen.shape == (C, D) and tgt.shape == (N,)

    nc = _get_nc()
    in_maps = []
    for c in range(N_CORES):
        xs = np.ascontiguousarray(x[c * ROWS : (c + 1) * ROWS])
        idxs = np.ascontiguousarray(tgt[c * ROWS : (c + 1) * ROWS].reshape(P, CHUNKS))
        in_maps.append({"x": xs, "idx": idxs, "centers": cen})

    res = run_bass_kernel_spmd(nc, in_maps, core_ids=list(range(N_CORES)))
    LAST_RESULTS = res

    total = 0.0
    for r in res.results:
        total += float(r["out"].astype(np.float64).sum())
    return np.array(0.5 * total, dtype=np.float32)


# revision 5
# speedup vs baseline: 1.1072x; 1.1072x over previous
"""CenterLoss kernel for Trainium2 (Bass/Tile), data-parallel over 8 NeuronCores.

loss = 0.5 * sum_i ||x_i - centers[targets_i]||^2

The reference materializes the full [N, C] distance matrix and gathers one
entry per row; here we gather only the target center rows (indirect DMA) and
do a fused subtract / square-accumulate, so the kernel is memory-bound on
~4 MB of HBM traffic per core instead of a 69 GFLOP matmul.

Sharding: inputs/targets split along batch N across 8 cores (512 rows each),
centers replicated. Each core returns 128 per-partition partial sums; host
sums them and scales by 0.5.
"""

import numpy as np

import concourse.bacc as bacc
import concourse.bass as bass
import concourse.tile as tile
from concourse import mybir
from concourse.bass_utils import run_bass_kernel_spmd

N, C, D = 4096, 8192, 1024
N_CORES = 8
ROWS = N // N_CORES  # 512 rows per core
P = 128              # SBUF partitions
CHUNKS = ROWS // P   # 4 chunks of 128 rows

# Stashed BassKernelResults from the most recent kernel() call (for profiling).
LAST_RESULTS = None
_NC_CACHE = None


def _build_bass():
    nc = bacc.Bacc("TRN2", target_bir_lowering=False)
    x = nc.dram_tensor("x", [ROWS, D], mybir.dt.float32, kind="ExternalInput")
    idx = nc.dram_tensor("idx", [P, CHUNKS], mybir.dt.int32, kind="ExternalInput")
    centers = nc.dram_tensor("centers", [C, D], mybir.dt.float32, kind="ExternalInput")
    out = nc.dram_tensor("out", [P, CHUNKS], mybir.dt.float32, kind="ExternalOutput")

    with tile.TileContext(nc) as tc:
        with (
            tc.tile_pool(name="io", bufs=1) as io,
            tc.tile_pool(name="cpool", bufs=CHUNKS) as cp,
            tc.tile_pool(name="small", bufs=1) as small,
        ):
            idx_sb = small.tile([P, CHUNKS], mybir.dt.int32)
            nc.sync.dma_start(idx_sb[:], idx[:, :])
            # Row r of the shard lives at partition p = r // CHUNKS, chunk
            # t = r % CHUNKS: one DMA, 16 KB contiguous per partition.
            xt = io.tile([P, CHUNKS * D], mybir.dt.float32)
            nc.sync.dma_start(xt[:], x.rearrange("(p t) d -> p (t d)", p=P))
            acc = small.tile([P, CHUNKS], mybir.dt.float32)
            for t in range(CHUNKS):
                ct = cp.tile([P, D], mybir.dt.float32, tag="c")
                nc.gpsimd.indirect_dma_start(
                    out=ct[:],
                    out_offset=None,
                    in_=centers[:, :],
                    in_offset=bass.IndirectOffsetOnAxis(
                        ap=idx_sb[:, t : t + 1], axis=0
                    ),
                )
                # d = x - c (in place over the gathered centers)
                nc.vector.tensor_sub(ct[:], xt[:, t * D : (t + 1) * D], ct[:])
                # acc[:, t] = sum_d d^2 (ACT engine, fused square + row-sum)
                nc.scalar.activation(
                    out=ct[:],
                    in_=ct[:],
                    func=mybir.ActivationFunctionType.Square,
                    accum_out=acc[:, t : t + 1],
                )
                # Ship each chunk's partials immediately so the HBM write
                # flush overlaps the remaining chunks' compute.
                nc.sync.dma_start(out[:, t : t + 1], acc[:, t : t + 1])
    nc.finalize()
    return nc


def _get_nc():
    global _NC_CACHE
    if _NC_CACHE is None:
        _NC_CACHE = _build_bass()
    return _NC_CACHE


def kernel(inputs, targets, centers):
    global LAST_RESULTS
    x = np.ascontiguousarray(np.asarray(inputs, dtype=np.float32))
    tgt = np.asarray(targets).astype(np.int32)
    cen = np.ascontiguousarray(np.asarray(centers, dtype=np.float32))
    assert x.shape == (N, D) and cen.shape == (C, D) and tgt.shape == (N,)

    nc = _get_nc()
    in_maps = []
    for c in range(N_CORES):
        xs = np.ascontiguousarray(x[c * ROWS : (c + 1) * ROWS])
        idxs = np.ascontiguousarray(tgt[c * ROWS : (c + 1) * ROWS].reshape(P, CHUNKS))
        in_maps.append({"x": xs, "idx": idxs, "centers": cen})

    res = run_bass_kernel_spmd(nc, in_maps, core_ids=list(range(N_CORES)))
    LAST_RESULTS = res

    total = 0.0
    for r in res.results:
        total += float(r["out"].astype(np.float64).sum())
    return np.array(0.5 * total, dtype=np.float32)


# revision 6
# speedup vs baseline: 1.2580x; 1.1361x over previous
"""CenterLoss kernel for Trainium2 (Bass/Tile), data-parallel over 8 NeuronCores.

loss = 0.5 * sum_i ||x_i - centers[targets_i]||^2

The reference materializes the full [N, C] distance matrix and gathers one
entry per row; here we gather only the target center rows (indirect DMA) and
do a fused subtract / square-accumulate, so the kernel is memory-bound on
~4 MB of HBM traffic per core instead of a 69 GFLOP matmul.

Sharding: inputs/targets split along batch N across 8 cores (512 rows each),
centers replicated. Each core returns 128 per-partition partial sums; host
sums them and scales by 0.5.
"""

import numpy as np

import concourse.bacc as bacc
import concourse.bass as bass
import concourse.tile as tile
from concourse import mybir
from concourse.bass_utils import run_bass_kernel_spmd

N, C, D = 4096, 8192, 1024
N_CORES = 8
ROWS = N // N_CORES  # 512 rows per core
P = 128              # SBUF partitions
CHUNKS = ROWS // P   # 4 chunks of 128 rows

# Stashed BassKernelResults from the most recent kernel() call (for profiling).
LAST_RESULTS = None
_NC_CACHE = None


def _build_bass():
    nc = bacc.Bacc("TRN2", target_bir_lowering=False)
    x = nc.dram_tensor("x", [ROWS, D], mybir.dt.float32, kind="ExternalInput")
    idx = nc.dram_tensor("idx", [P, CHUNKS], mybir.dt.int32, kind="ExternalInput")
    centers = nc.dram_tensor("centers", [C, D], mybir.dt.float32, kind="ExternalInput")
    out = nc.dram_tensor("out", [1, CHUNKS], mybir.dt.float32, kind="ExternalOutput")

    with tile.TileContext(nc) as tc:
        with (
            tc.tile_pool(name="io", bufs=1) as io,
            tc.tile_pool(name="cpool", bufs=CHUNKS) as cp,
            tc.tile_pool(name="psum", bufs=1, space="PSUM") as pp,
            tc.tile_pool(name="small", bufs=1) as small,
        ):
            idx_sb = small.tile([P, CHUNKS], mybir.dt.int32)
            nc.sync.dma_start(idx_sb[:], idx[:, :])
            # Row r of the shard lives at partition p = r // CHUNKS, chunk
            # t = r % CHUNKS: one DMA, 16 KB contiguous per partition.
            xt = io.tile([P, CHUNKS * D], mybir.dt.float32)
            nc.sync.dma_start(xt[:], x.rearrange("(p t) d -> p (t d)", p=P))
            ones = small.tile([P, 1], mybir.dt.float32)
            nc.vector.memset(ones[:], 1.0)
            acc = small.tile([P, CHUNKS], mybir.dt.float32)
            for t in range(CHUNKS):
                ct = cp.tile([P, D], mybir.dt.float32, tag="c")
                nc.gpsimd.indirect_dma_start(
                    out=ct[:],
                    out_offset=None,
                    in_=centers[:, :],
                    in_offset=bass.IndirectOffsetOnAxis(
                        ap=idx_sb[:, t : t + 1], axis=0
                    ),
                )
                # d = x - c (in place over the gathered centers)
                nc.vector.tensor_sub(ct[:], xt[:, t * D : (t + 1) * D], ct[:])
                # acc[:, t] = sum_d d^2 (ACT engine, fused square + row-sum)
                nc.scalar.activation(
                    out=ct[:],
                    in_=ct[:],
                    func=mybir.ActivationFunctionType.Square,
                    accum_out=acc[:, t : t + 1],
                )
            # Partition-reduce on the (idle) PE: ones^T @ acc = [1, CHUNKS],
            # so the output DMA is a single 16-byte descriptor — its HBM
            # write-ack flush is one engine instead of sixteen.
            psum = pp.tile([1, CHUNKS], mybir.dt.float32)
            nc.tensor.matmul(psum[:], lhsT=ones[:], rhs=acc[:], start=True, stop=True)
            res = small.tile([1, CHUNKS], mybir.dt.float32)
            nc.vector.tensor_copy(res[:], psum[:])
            nc.sync.dma_start(out[:, :], res[:])
    nc.finalize()
    return nc


def _get_nc():
    global _NC_CACHE
    if _NC_CACHE is None:
        _NC_CACHE = _build_bass()
    return _NC_CACHE


def kernel(inputs, targets, centers):
    global LAST_RESULTS
    x = np.ascontiguousarray(np.asarray(inputs, dtype=np.float32))
    tgt = np.asarray(targets).astype(np.int32)
    cen = np.ascontiguousarray(np.asarray(centers, dtype=np.float32))
    assert x.shape == (N, D) and cen.shape == (C, D) and tgt.shape == (N,)

    nc = _get_nc()
    in_maps = []
    for c in range(N_CORES):
        xs = np.ascontiguousarray(x[c * ROWS : (c + 1) * ROWS])
        idxs = np.ascontiguousarray(tgt[c * ROWS : (c + 1) * ROWS].reshape(P, CHUNKS))
        in_maps.append({"x": xs, "idx": idxs, "centers": cen})

    res = run_bass_kernel_spmd(nc, in_maps, core_ids=list(range(N_CORES)))
    LAST_RESULTS = res

    total = 0.0
    for r in res.results:
        total += float(r["out"].astype(np.float64).sum())
    return np.array(0.5 * total, dtype=np.float32)
